# revision 24
# baseline (speedup 1.0000x reference)
"""MobileMQA Trainium2 kernel v2 (8 NeuronCores, SPMD).

Reference computation (per batch b of 2):
  q  = x @ wq + bq                         [1024 tok, 512]
  kv = x @ wkv + bkv                       [1024 tok, 1024]
  kv = depthwise3x3_s2_same(kv) + dw_bias  [256 sp, 1024]
  k, v = split(kv)  -> shared-KV length M=2048 (channel fold)
  attn = softmax(q @ k^T * 0.125); out = attn @ v
  y = out @ wo + bo

Sharding: core c handles batch b=c//4, query chunk j=c%4 (256 tokens).
KV path (proj+conv) replicated across the 4 cores of a batch (MQA).

v2 design vs baseline:
  - fp16 everywhere on the PE (1 cycle/row at any N; 11-bit mantissa).
  - Fused P1 (proj+conv) / P2 (attention) pipeline: attention units for
    ch-tile t are emitted interleaved with ch-tile t+1's proj/conv so the
    PE never waits for the serial ACT exp chain.
  - AV matmul flipped: exp(S) chunks are the stationary operand, V_aug
    [128m, 65] the moving one -> 65-row streams instead of 512.
  - Scores single-half (kT2/qT2 [64, 2048], no partition duplication).
  - Conv diagonal weight matrices are host-built and DMA'd (fp16), bias
    plane added in-place into conv PSUM, so K/V tiles are plain copies.
  - Normalization via per-partition 1/z scalar (z = ones-column of AV).
"""
import sys

for _p in ("/opt/trn_rl_repo", "/opt/trn_rl_repo/concourse"):
    if _p not in sys.path:
        sys.path.insert(0, _p)

import numpy as np

import concourse.bass as bass
import concourse.mybir as mybir
import concourse.tile as tile
from concourse import bacc
from concourse.bass_utils import run_bass_kernel_spmd
from concourse.masks import make_identity

F32 = mybir.dt.float32
F16 = mybir.dt.float16
BF16 = mybir.dt.bfloat16
AF = mybir.ActivationFunctionType
ALU = mybir.AluOpType

DIM = 512
NH = 8
HD = 64
B, H, W = 2, 32, 32
L = H * W            # 1024 tokens per batch
KH = KW = 16
NS = KH * KW         # 256 conv-output spatial positions
M = NS * NH          # 2048 shared-KV positions
CH = 2 * DIM         # 1024 kv channels
SCALE = HD ** -0.5   # 0.125
PADW = 33            # padded conv input row (32 + 1 SAME pad after)
NPAD = PADW * PADW   # 1089

_NC_CACHE = {}


def _build_program():
    nc = bacc.Bacc(None)

    xT_d = nc.dram_tensor("xT", [DIM, L], F16, kind="ExternalInput")
    xTc_d = nc.dram_tensor("xTc", [DIM, 256], F16, kind="ExternalInput")
    wkv_d = nc.dram_tensor("wkv", [DIM, CH], F16, kind="ExternalInput")
    wq_d = nc.dram_tensor("wq", [DIM, DIM], F16, kind="ExternalInput")
    wo_d = nc.dram_tensor("wo", [DIM, DIM], F16, kind="ExternalInput")
    bpl_d = nc.dram_tensor("bpl", [CH, NS], F16, kind="ExternalInput")
    dgw_d = nc.dram_tensor("dgw", [128, 40 * 128], F16, kind="ExternalInput")
    # cst cols: 0-3 bq tiles, 4-7 bo tiles
    cst_d = nc.dram_tensor("cst", [128, 40], F32, kind="ExternalInput")
    y_d = nc.dram_tensor("y", [DIM, 256], F32, kind="ExternalOutput")
    import os as _os
    _dbg = _os.environ.get("BASSDBG") == "1"
    if _dbg:
        kT2_o = nc.dram_tensor("kT2o", [64, M], F16, kind="ExternalOutput")
        qT2_o = nc.dram_tensor("qT2o", [64, M], F16, kind="ExternalOutput")
        vaug_o = nc.dram_tensor("vaugo", [128, 16 * (HD + 1)], BF16,
                                kind="ExternalOutput")
        asb_o = nc.dram_tensor("asbo", [128, 16 * HD], F16,
                               kind="ExternalOutput")

    with tile.TileContext(nc) as tc:
        with tc.tile_pool(name="wp", bufs=1) as wp, \
             tc.tile_pool(name="expp", bufs=8) as expp, \
             tc.tile_pool(name="tmpq", bufs=2) as tmppool, \
             tc.tile_pool(name="ps", bufs=1, space="PSUM") as ps:

            # ---------------- input DMAs (priority order) ----------------
            cst = wp.tile([128, 40], F32, tag="cst")

            xTc = wp.tile([128, 4, 256], F16, tag="xTc")
            xTc_r = xTc_d[:, :].rearrange("(k p) t -> p k t", p=128)
            wq = wp.tile([128, 4, DIM], F16, tag="wq")
            wq_r = wq_d[:, :].rearrange("(k p) c -> p k c", p=128)
            xT = wp.tile([128, 4, L], F16, tag="xT")
            wkv = wp.tile([128, 4, CH], F16, tag="wkv")
            dgw = wp.tile([128, 40, 128], F16, tag="dgw")
            bpl = wp.tile([128, 8, NS], F16, tag="bpl")
            xT_r = xT_d[:, :].rearrange("(k p) t -> p k t", p=128)
            wkv_r = wkv_d[:, :].rearrange("(k p) c -> p k c", p=128)
            bpl_r = bpl_d[:, :].rearrange("(t p) s -> p t s", p=128)

            # critical-path order: K0/V0 proj operands, then q operands
            nc.sync.dma_start(out=wkv[:, :, 0:256], in_=wkv_r[:, :, 0:256])
            nc.sync.dma_start(out=xT[:, 0:2, 0:512], in_=xT_r[:, 0:2, 0:512])
            nc.sync.dma_start(out=xT[:, 2:4, 0:512], in_=xT_r[:, 2:4, 0:512])
            nc.sync.dma_start(out=xTc, in_=xTc_r)
            nc.sync.dma_start(out=wq, in_=wq_r)
            nc.sync.dma_start(out=cst, in_=cst_d[:, :])
            nc.sync.dma_start(out=xT[:, :, 512:L], in_=xT_r[:, :, 512:L])
            nc.sync.dma_start(out=dgw[:, 0:10, :],
                              in_=dgw_d[:, 0:10 * 128])
            nc.sync.dma_start(out=bpl[:, 0:2, :], in_=bpl_r[:, 0:2, :])
            for dg in range(2, 8):
                nc.sync.dma_start(out=wkv[:, :, dg * 128:(dg + 1) * 128],
                                  in_=wkv_r[:, :, dg * 128:(dg + 1) * 128])
            nc.sync.dma_start(out=dgw[:, 10:25, :],
                              in_=dgw_d[:, 10 * 128:25 * 128])
            nc.sync.dma_start(out=dgw[:, 25:40, :],
                              in_=dgw_d[:, 25 * 128:40 * 128])
            nc.sync.dma_start(out=bpl[:, 2:8, :], in_=bpl_r[:, 2:8, :])
            wo = wp.tile([128, 4, DIM], F16, tag="wo")
            nc.sync.dma_start(out=wo,
                              in_=wo_d[:, :].rearrange("(k p) c -> p k c", p=128))

            # ---------------- persistent SBUF state ----------------
            ident = wp.tile([128, 128], F16, tag="ident")
            make_identity(nc, ident)
            # preload exp ACT table during the DMA window
            warm = wp.tile([1, 1], F32, tag="warm")
            nc.vector.memset(warm, 0.0)
            nc.scalar.activation(warm[:, :], warm[:, :], AF.Exp)

            kT2 = wp.tile([64, M], F16, tag="kT2")
            qT2 = wp.tile([64, M], F16, tag="qT2")
            vaug = wp.tile([128, 16, HD + 1], BF16, tag="vaug")
            nc.vector.memset(vaug[:, :, HD:HD + 1], 1.0)
            attnT = wp.tile([128, 4, 256], F16, tag="attnT")
            attn_sb = wp.tile([128, 16, HD], F16, tag="attn_sb")
            zr = wp.tile([128, 16], F32, tag="zr")
            vacc = wp.tile([128, NS], F16, tag="vacc")
            ysb = wp.tile([128, 4, 256], F32, tag="ysb")

            # two explicit conv-input buffers; pad cells zeroed once
            kvsb = []
            for i in range(2):
                kb = wp.tile([128, NPAD], F16, tag=f"kvsb{i}",
                             name=f"kvsb{i}")
                nc.vector.memset(kb[:, :], 0.0)
                kvsb.append(kb)

            # PE warm-up: keep the array busy through the DMA window so the
            # p-state ramp completes before the real matmuls arrive.
            for _w in range(5):
                wmm = ps.tile([128, 512], F32, tag="kvp", bufs=1)
                nc.tensor.matmul(wmm[:, :], ident[:, :], kvsb[0][:, 0:512],
                                 start=True, stop=True)

            # ---------------- helpers ----------------
            def kv_proj_half(dg, nh):
                """kv proj for ch-tile dg, token half nh -> PSUM [128, 512]."""
                kvp = ps.tile([128, 512], F32, tag="kvp", bufs=1)
                for k in range(4):
                    nc.tensor.matmul(kvp[:, :],
                                     wkv[:, k, dg * 128:(dg + 1) * 128],
                                     xT[:, k, nh * 512:(nh + 1) * 512],
                                     start=(k == 0), stop=(k == 3))
                return kvp

            def pad_copy(dg, nh, kvp, eng):
                kb = kvsb[dg % 2]
                dst = bass.AP(tensor=kb.tensor,
                              offset=kb.offset + PADW * 16 * nh,
                              ap=[kb.ap[0], [PADW, 16], [1, 32]])
                src = kvp[:, :].rearrange("p (a b) -> p a b", b=32)
                if eng is nc.scalar:
                    eng.copy(dst, src)
                else:
                    eng.tensor_copy(dst, src)

            tmpp = tmppool

            def conv(dg):
                """Conv taps 0-6 on PE (PSUM cvp); taps 7,8 + bias into an
                SBUF side-accumulator (DVE then gpsimd), merged at the
                K/V stt."""
                kb = kvsb[dg % 2]
                tmp = tmpp.tile([128, NS], F32, tag="tmp")
                for tap, eng in ((5, nc.vector), (6, nc.vector),
                                 (7, nc.vector), (8, nc.vector)):
                    dy, dx = tap // 3, tap % 3
                    win = bass.AP(tensor=kb.tensor,
                                  offset=kb.offset + PADW * dy + dx,
                                  ap=[kb.ap[0], [2 * PADW, KH], [2, KW]])
                    col = 8 + (tap - 5) * 8 + dg
                    in1 = bpl[:, dg, :] if tap == 5 else tmp[:, :]
                    eng.scalar_tensor_tensor(
                        tmp[:, :], win, cst[:, col:col + 1], in1,
                        op0=ALU.mult, op1=ALU.add)
                cvp = ps.tile([128, NS], F32, tag="kvp", bufs=1)
                for tap in range(5):
                    dy, dx = tap // 3, tap % 3
                    win = bass.AP(tensor=kb.tensor,
                                  offset=kb.offset + PADW * dy + dx,
                                  ap=[kb.ap[0], [2 * PADW, KH], [2, KW]])
                    nc.tensor.matmul(cvp[:, :], dgw[:, dg * 5 + tap, :], win,
                                     start=(tap == 0), stop=(tap == 4))
                return cvp, tmp

            def k_tile(t, cvp, tmp):
                # gpsimd cannot read PSUM: merge to SBUF, then cross-copy
                kacc = tmpp.tile([128, NS], F16, tag="kacc")
                nc.vector.scalar_tensor_tensor(
                    kacc[:, :], cvp[:, :], 1.0, tmp[:, :],
                    op0=ALU.mult, op1=ALU.add)
                nc.gpsimd.tensor_copy(kT2[:, (2 * t) * 256:(2 * t + 1) * 256],
                                      kacc[0:64, :])
                nc.gpsimd.tensor_copy(
                    kT2[:, (2 * t + 1) * 256:(2 * t + 2) * 256],
                    kacc[64:128, :])

            def v_tile(t, cvp, tmp):
                nc.vector.scalar_tensor_tensor(
                    vacc[:, :], cvp[:, :], 1.0, tmp[:, :],
                    op0=ALU.mult, op1=ALU.add)
                for gi in range(2):
                    vt = ps.tile([128, 128], F16, tag="kvp", bufs=1)
                    for sh in range(2):
                        nc.tensor.transpose(
                            vt[:, sh * 64:(sh + 1) * 64],
                            vacc[gi * 64:(gi + 1) * 64,
                                 sh * 128:(sh + 1) * 128],
                            ident[gi * 64:(gi + 1) * 64,
                                  gi * 64:(gi + 1) * 64])
                    mt0 = 4 * t + 2 * gi
                    nc.vector.tensor_copy(
                        vaug[:, mt0:mt0 + 2, 0:HD],
                        vt[:, :].rearrange("p (a b) -> p a b", b=64))

            av = ps.tile([128, 16, HD + 1], F32, tag="av", bufs=1)
            # pre-zero the whole av region (PSUM zero-regions are 2KB; the
            # 16 interleaved 260B accumulation groups must share one group)
            av_flat = bass.AP(tensor=av.tensor, offset=av.offset,
                              ap=[av.ap[0], [1, 16 * (HD + 1)]])
            for z0 in (0, 512):
                zmm = bass.AP(tensor=av.tensor, offset=av.offset + z0,
                              ap=[av.ap[0], [1, 512]])
                nc.tensor.matmul(zmm, ident[:, :], kvsb[0][:, 0:512],
                                 start=True, stop=True, skip_group_check=True)
            zmm2 = bass.AP(tensor=av.tensor, offset=av.offset + 1024,
                           ap=[av.ap[0], [1, 16]])
            nc.tensor.matmul(zmm2, ident[:, :], kvsb[0][:, 0:16],
                             start=True, stop=True, skip_group_check=True)

            def sc_unit(mt, uq):
                st = ps.tile([128, 512], F32, tag="st", bufs=4)
                nc.tensor.matmul(st[:, :],
                                 kT2[:, mt * 128:(mt + 1) * 128],
                                 qT2[:, uq * 512:(uq + 1) * 512],
                                 start=True, stop=True)
                ex = expp.tile([128, 512], BF16, tag="ex")
                nc.scalar.activation(ex[:, :], st[:, :], AF.Exp,
                                     scale=float(SCALE))
                return ex

            def av_unit(mt, uq, ex):
                for j in range(4):
                    nc.tensor.matmul(av[:, uq * 4 + j, :],
                                     ex[:, j * 128:(j + 1) * 128],
                                     vaug[:, mt, :],
                                     start=False, stop=(mt == 15),
                                     skip_group_check=True)

            # ---------------- q projection (emitted via emit_qproj) --------
            def emit_qproj(t):
                qp = ps.tile([128, 512], F32, tag="st", bufs=4)
                qpv = qp[:, 0:256]
                for k in range(4):
                    nc.tensor.matmul(qpv, wq[:, k, t * 128:(t + 1) * 128],
                                     xTc[:, k, :],
                                     start=(k == 0), stop=(k == 3))
                nc.vector.tensor_scalar_add(qpv, qpv, cst[:, t:t + 1])
                nc.vector.tensor_copy(qT2[:, (2 * t) * 256:(2 * t + 1) * 256],
                                      qp[0:64, 0:256])
                qstage = tmpp.tile([128, 256], F16, tag="kacc", name="qstage")
                nc.vector.tensor_copy(qstage[64:128, :], qp[64:128, 0:256])
                nc.gpsimd.tensor_copy(qT2[:, (2 * t + 1) * 256:(2 * t + 2) * 256],
                                      qstage[64:128, :])

            # ---------------- fused P1/P2 pipeline ----------------
            # K-proj halves rotate through the "st" slots (parallel to the
            # V chain on "kvp"); t=0 pads on ACT (idle pre-exp), later K
            # pads on DVE, V pads on gpsimd.
            kvp_store = {}

            def pad_eng(dg):
                # gpsimd cannot access PSUM; split PSUM reads DVE/ACT
                if dg < 2:
                    return nc.scalar
                return nc.vector

            def proj_op(dg, nh):
                if dg % 2 == 0:  # K tile -> st slots
                    kvp = ps.tile([128, 512], F32, tag="st", bufs=4,
                                  name="kvpk")
                else:
                    kvp = ps.tile([128, 512], F32, tag="kvp", bufs=1)
                for k in range(4):
                    nc.tensor.matmul(kvp[:, :],
                                     wkv[:, k, dg * 128:(dg + 1) * 128],
                                     xT[:, k, nh * 512:(nh + 1) * 512],
                                     start=(k == 0), stop=(k == 3))
                kvp_store[(dg, nh)] = kvp

            def pad_op(dg, nh):
                pad_copy(dg, nh, kvp_store.pop((dg, nh)), pad_eng(dg))

            def p1_ops(t):
                dgk, dgv = 2 * t, 2 * t + 1
                mk = lambda f, *a: (lambda: f(*a))
                cstore = {}

                def conv_op(dg):
                    cstore[dg] = conv(dg)

                def ktail(tt):
                    k_tile(tt, *cstore.pop(2 * tt))

                def vtail(tt):
                    v_tile(tt, *cstore.pop(2 * tt + 1))

                return [
                    mk(proj_op, dgk, 0), mk(pad_op, dgk, 0),
                    mk(proj_op, dgv, 0), mk(pad_op, dgv, 0),
                    mk(proj_op, dgk, 1), mk(pad_op, dgk, 1),
                    mk(proj_op, dgv, 1), mk(pad_op, dgv, 1),
                    mk(conv_op, dgk), mk(ktail, t),
                    mk(conv_op, dgv), mk(vtail, t),
                ]

            ops0 = p1_ops(0)
            for op in ops0[0:4]:     # K0/V0 first halves
                op()
            emit_qproj(0)
            for op in ops0[4:8]:     # second halves
                op()
            emit_qproj(1)
            emit_qproj(2)
            for op in ops0[8:10]:    # conv K0
                op()
            emit_qproj(3)
            for op in ops0[10:12]:   # conv V0
                op()
            # Unit stream with av lagging sc by one; p1 micro-ops of tile
            # t+1 round-robined between tile t's units.
            pending = []
            for t in range(4):
                us = [(mt, uq) for mt in range(4 * t, 4 * t + 4)
                      for uq in range(4)]
                chunks = p1_ops(t + 1) if t < 3 else []
                nu, nch = len(us), len(chunks)
                ci = 0
                for i, u in enumerate(us):
                    mt, uq = u
                    ex = sc_unit(mt, uq)
                    pending.append((mt, uq, ex))
                    if len(pending) > 2:
                        av_unit(*pending.pop(0))
                    want = ((i + 1) * nch) // nu
                    while ci < want:
                        chunks[ci]()
                        ci += 1
            for p_ in pending:
                av_unit(*p_)

            # ---------------- normalize + attnT + y proj ----------------
            # chunks 0-7 (half 0) complete one unit before 8-15; normalize,
            # transpose and accumulate y per ch-block as results land.
            def slot_of(chunk):
                h, lh = chunk // 2, chunk % 2
                return 4 * (h // 2) + 2 * lh + (h % 2)

            def norm_chunk(chunk):
                slot = slot_of(chunk)
                nc.vector.reciprocal(zr[:, chunk:chunk + 1],
                                     av[:, chunk, HD:HD + 1])
                if chunk % 2 == 0:
                    nc.vector.tensor_scalar_mul(attn_sb[:, slot, :],
                                                av[:, chunk, 0:HD],
                                                zr[:, chunk:chunk + 1])
                else:
                    nc.scalar.mul(attn_sb[:, slot, :], av[:, chunk, 0:HD],
                                  zr[:, chunk:chunk + 1])

            yps = [None, None]

            def attnT_block(kk):
                for lh in range(2):
                    vt2 = ps.tile([128, 128], F16, tag="kvp", bufs=1)
                    s0 = 4 * kk + 2 * lh
                    src_ap = bass.AP(
                        tensor=attn_sb.tensor,
                        offset=attn_sb.offset + s0 * HD,
                        ap=[attn_sb.ap[0], [1, 128]])
                    nc.tensor.transpose(vt2[:, :], src_ap, ident[:, :])
                    if kk < 2:
                        nc.scalar.copy(
                            attnT[:, kk, lh * 128:(lh + 1) * 128], vt2[:, :])
                    else:
                        nc.vector.tensor_copy(
                            attnT[:, kk, lh * 128:(lh + 1) * 128], vt2[:, :])
                for m in range(4):
                    if yps[m // 2] is None:
                        yps[m // 2] = ps.tile([128, 512], F32, tag="st",
                                              bufs=4, name=f"yp{m // 2}")
                    nc.tensor.matmul(
                        yps[m // 2][:, (m % 2) * 256:(m % 2) * 256 + 256],
                        wo[:, kk, m * 128:(m + 1) * 128],
                        attnT[:, kk, :],
                        start=(kk == 0 and m % 2 == 0), stop=(kk == 3),
                        skip_group_check=True)

            for chunk in range(8):
                norm_chunk(chunk)
            attnT_block(0)
            attnT_block(1)
            for chunk in range(8, 16):
                norm_chunk(chunk)
            attnT_block(2)
            attnT_block(3)
            for m in range(4):
                ypv = yps[m // 2][:, (m % 2) * 256:(m % 2) * 256 + 256]
                if m < 2:
                    nc.vector.tensor_scalar_add(ysb[:, m, :], ypv,
                                                cst[:, 4 + m:5 + m])
                else:
                    nc.scalar.add(ysb[:, m, :], ypv, cst[:, 4 + m:5 + m])
            if _dbg:
                nc.sync.dma_start(out=kT2_o[:, :], in_=kT2[:, :])
                nc.sync.dma_start(out=qT2_o[:, :], in_=qT2[:, :])
                nc.sync.dma_start(out=vaug_o[:, :],
                                  in_=vaug[:, :, :].rearrange("p a b -> p (a b)"))
                nc.sync.dma_start(out=asb_o[:, :],
                                  in_=attn_sb[:, :, :].rearrange("p a b -> p (a b)"))
            y_r = y_d[:, :].rearrange("(m p) t -> p m t", p=128)
            nc.sync.dma_start(out=y_d[0:256, :].rearrange("(m p) t -> p m t", p=128),
                              in_=ysb[:, 0:2, :])
            nc.sync.dma_start(out=y_d[256:512, :].rearrange("(m p) t -> p m t", p=128),
                              in_=ysb[:, 2:4, :])

    nc.finalize()
    return nc


def _get_program():
    if "nc" not in _NC_CACHE:
        _NC_CACHE["nc"] = _build_program()
    return _NC_CACHE["nc"]


def _host_prep(x, wq, bq, wkv, bkv, dw_kernel, dw_bias, wo, bo):
    """Build the 8 per-core input maps."""
    x = np.ascontiguousarray(np.asarray(x, np.float32))
    wq16 = np.asarray(wq, np.float32).astype(np.float16)
    wo16 = np.asarray(wo, np.float32).astype(np.float16)
    bq = np.asarray(bq, np.float32)
    bkv = np.asarray(bkv, np.float32)
    dw_bias = np.asarray(dw_bias, np.float32)
    bo = np.asarray(bo, np.float32)
    dww = np.asarray(dw_kernel, np.float32).reshape(9, CH).T.copy()  # [1024, 9]

    # channel-tile processing order K0 V0 K1 V1 ... ; dg -> channel base
    dg_base = []
    for t in range(4):
        dg_base += [t * 128, DIM + t * 128]

    # wkv columns reordered to dg order
    wkv_f = np.asarray(wkv, np.float32)
    wkv16 = np.empty((DIM, CH), np.float16)
    for dg in range(8):
        b0 = dg_base[dg]
        wkv16[:, dg * 128:(dg + 1) * 128] = wkv_f[:, b0:b0 + 128]

    # bias plane: dw_bias + bkv * sum(valid taps), SAME padding aware,
    # rows in dg order
    oy = np.arange(KH)
    valid_y = (2 * oy[:, None] + np.arange(3)[None, :]) < H      # [16, 3]
    valid_x = valid_y.copy()
    wsum = np.zeros((CH, KH, KW), np.float32)
    for tap in range(9):
        dy, dx = tap // 3, tap % 3
        m2 = np.outer(valid_y[:, dy], valid_x[:, dx]).astype(np.float32)
        wsum += dww[:, tap][:, None, None] * m2[None, :, :]
    bpl_full = (dw_bias[:, None] + bkv[:, None] * wsum.reshape(CH, NS))
    bpl16 = np.empty((CH, NS), np.float16)
    for dg in range(8):
        b0 = dg_base[dg]
        bpl16[dg * 128:(dg + 1) * 128] = bpl_full[b0:b0 + 128]

    # conv diagonal weights [128, 72*128] fp16, dg-major then tap
    dgw = np.zeros((128, 40, 128), np.float16)
    idx = np.arange(128)
    for dg in range(8):
        b0 = dg_base[dg]
        for tap in range(5):
            dgw[idx, dg * 5 + tap, idx] = dww[b0 + idx, tap].astype(np.float16)
    dgw = dgw.reshape(128, 40 * 128)

    cst = np.zeros((128, 40), np.float32)
    cst[:, 0:4] = bq.reshape(4, 128).T
    cst[:, 4:8] = bo.reshape(4, 128).T
    for dg in range(8):
        b0 = dg_base[dg]
        for tap in range(5, 9):
            cst[:, 8 + (tap - 5) * 8 + dg] = dww[b0:b0 + 128, tap]

    in_maps = []
    for c in range(8):
        b, j = c // 4, c % 4
        xT = x[b].reshape(L, DIM).T.astype(np.float16)
        in_maps.append({
            "xT": np.ascontiguousarray(xT),
            "xTc": np.ascontiguousarray(xT[:, j * 256:(j + 1) * 256]),
            "wkv": wkv16, "wq": wq16, "wo": wo16,
            "bpl": bpl16, "dgw": dgw, "cst": cst,
        })
    return in_maps


def kernel(**inputs) -> np.ndarray:
    nc = _get_program()
    in_maps = _host_prep(**inputs)
    res = run_bass_kernel_spmd(nc, in_maps, core_ids=list(range(8)))
    out = np.zeros((B, H, W, DIM), np.float32)
    flat = out.reshape(B, L, DIM)
    for c in range(8):
        b, j = c // 4, c % 4
        flat[b, j * 256:(j + 1) * 256, :] = res.results[c]["y"].T
    return out


# revision 25
# speedup vs baseline: 1.0112x; 1.0112x over previous
"""MobileMQA Trainium2 kernel v2 (8 NeuronCores, SPMD).

Reference computation (per batch b of 2):
  q  = x @ wq + bq                         [1024 tok, 512]
  kv = x @ wkv + bkv                       [1024 tok, 1024]
  kv = depthwise3x3_s2_same(kv) + dw_bias  [256 sp, 1024]
  k, v = split(kv)  -> shared-KV length M=2048 (channel fold)
  attn = softmax(q @ k^T * 0.125); out = attn @ v
  y = out @ wo + bo

Sharding: core c handles batch b=c//4, query chunk j=c%4 (256 tokens).
KV path (proj+conv) replicated across the 4 cores of a batch (MQA).

v2 design vs baseline:
  - fp16 everywhere on the PE (1 cycle/row at any N; 11-bit mantissa).
  - Fused P1 (proj+conv) / P2 (attention) pipeline: attention units for
    ch-tile t are emitted interleaved with ch-tile t+1's proj/conv so the
    PE never waits for the serial ACT exp chain.
  - AV matmul flipped: exp(S) chunks are the stationary operand, V_aug
    [128m, 65] the moving one -> 65-row streams instead of 512.
  - Scores single-half (kT2/qT2 [64, 2048], no partition duplication).
  - Conv diagonal weight matrices are host-built and DMA'd (fp16), bias
    plane added in-place into conv PSUM, so K/V tiles are plain copies.
  - Normalization via per-partition 1/z scalar (z = ones-column of AV).
"""
import sys

for _p in ("/opt/trn_rl_repo", "/opt/trn_rl_repo/concourse"):
    if _p not in sys.path:
        sys.path.insert(0, _p)

import numpy as np

import concourse.bass as bass
import concourse.mybir as mybir
import concourse.tile as tile
from concourse import bacc
from concourse.bass_utils import run_bass_kernel_spmd
from concourse.masks import make_identity

F32 = mybir.dt.float32
F16 = mybir.dt.float16
BF16 = mybir.dt.bfloat16
AF = mybir.ActivationFunctionType
ALU = mybir.AluOpType

DIM = 512
NH = 8
HD = 64
B, H, W = 2, 32, 32
L = H * W            # 1024 tokens per batch
KH = KW = 16
NS = KH * KW         # 256 conv-output spatial positions
M = NS * NH          # 2048 shared-KV positions
CH = 2 * DIM         # 1024 kv channels
SCALE = HD ** -0.5   # 0.125
PADW = 33            # padded conv input row (32 + 1 SAME pad after)
NPAD = PADW * PADW   # 1089

_NC_CACHE = {}


def _build_program():
    nc = bacc.Bacc(None)

    xT_d = nc.dram_tensor("xT", [DIM, L], F16, kind="ExternalInput")
    xTc_d = nc.dram_tensor("xTc", [DIM, 256], F16, kind="ExternalInput")
    wkv_d = nc.dram_tensor("wkv", [DIM, CH], F16, kind="ExternalInput")
    wq_d = nc.dram_tensor("wq", [DIM, DIM], F16, kind="ExternalInput")
    wo_d = nc.dram_tensor("wo", [DIM, DIM], F16, kind="ExternalInput")
    bpl_d = nc.dram_tensor("bpl", [CH, NS], F16, kind="ExternalInput")
    dgw_d = nc.dram_tensor("dgw", [128, 56 * 128], F16, kind="ExternalInput")
    # cst cols: 0-3 bq tiles, 4-7 bo tiles
    cst_d = nc.dram_tensor("cst", [128, 40], F32, kind="ExternalInput")
    y_d = nc.dram_tensor("y", [DIM, 256], F32, kind="ExternalOutput")
    import os as _os
    _dbg = _os.environ.get("BASSDBG") == "1"
    if _dbg:
        kT2_o = nc.dram_tensor("kT2o", [64, M], F16, kind="ExternalOutput")
        qT2_o = nc.dram_tensor("qT2o", [64, M], F16, kind="ExternalOutput")
        vaug_o = nc.dram_tensor("vaugo", [128, 16 * (HD + 1)], BF16,
                                kind="ExternalOutput")
        asb_o = nc.dram_tensor("asbo", [128, 16 * HD], F16,
                               kind="ExternalOutput")

    with tile.TileContext(nc) as tc:
        with tc.tile_pool(name="wp", bufs=1) as wp, \
             tc.tile_pool(name="expp", bufs=8) as expp, \
             tc.tile_pool(name="tmpq", bufs=2) as tmppool, \
             tc.tile_pool(name="ps", bufs=1, space="PSUM") as ps:

            # ---------------- input DMAs (priority order) ----------------
            cst = wp.tile([128, 40], F32, tag="cst")

            xTc = wp.tile([128, 4, 256], F16, tag="xTc")
            xTc_r = xTc_d[:, :].rearrange("(k p) t -> p k t", p=128)
            wq = wp.tile([128, 4, DIM], F16, tag="wq")
            wq_r = wq_d[:, :].rearrange("(k p) c -> p k c", p=128)
            xT = wp.tile([128, 4, L], F16, tag="xT")
            wkv = wp.tile([128, 4, CH], F16, tag="wkv")
            dgw = wp.tile([128, 56, 128], F16, tag="dgw")
            bpl = wp.tile([128, 8, NS], F16, tag="bpl")
            xT_r = xT_d[:, :].rearrange("(k p) t -> p k t", p=128)
            wkv_r = wkv_d[:, :].rearrange("(k p) c -> p k c", p=128)
            bpl_r = bpl_d[:, :].rearrange("(t p) s -> p t s", p=128)

            # critical-path order: K0/V0 proj operands, then q operands
            nc.sync.dma_start(out=wkv[:, :, 0:256], in_=wkv_r[:, :, 0:256])
            nc.sync.dma_start(out=xT[:, 0:2, 0:512], in_=xT_r[:, 0:2, 0:512])
            nc.sync.dma_start(out=xT[:, 2:4, 0:512], in_=xT_r[:, 2:4, 0:512])
            nc.sync.dma_start(out=xTc, in_=xTc_r)
            nc.sync.dma_start(out=wq, in_=wq_r)
            nc.sync.dma_start(out=cst, in_=cst_d[:, :])
            nc.sync.dma_start(out=xT[:, :, 512:L], in_=xT_r[:, :, 512:L])
            nc.sync.dma_start(out=dgw[:, 0:14, :],
                              in_=dgw_d[:, 0:14 * 128])
            nc.sync.dma_start(out=bpl[:, 0:2, :], in_=bpl_r[:, 0:2, :])
            for dg in range(2, 8):
                nc.sync.dma_start(out=wkv[:, :, dg * 128:(dg + 1) * 128],
                                  in_=wkv_r[:, :, dg * 128:(dg + 1) * 128])
            nc.sync.dma_start(out=dgw[:, 14:35, :],
                              in_=dgw_d[:, 14 * 128:35 * 128])
            nc.sync.dma_start(out=dgw[:, 35:56, :],
                              in_=dgw_d[:, 35 * 128:56 * 128])
            nc.sync.dma_start(out=bpl[:, 2:8, :], in_=bpl_r[:, 2:8, :])
            wo = wp.tile([128, 4, DIM], F16, tag="wo")
            nc.sync.dma_start(out=wo,
                              in_=wo_d[:, :].rearrange("(k p) c -> p k c", p=128))

            # ---------------- persistent SBUF state ----------------
            ident = wp.tile([128, 128], F16, tag="ident")
            make_identity(nc, ident)
            # preload exp ACT table during the DMA window
            warm = wp.tile([1, 1], F32, tag="warm")
            nc.vector.memset(warm, 0.0)
            nc.scalar.activation(warm[:, :], warm[:, :], AF.Exp)

            kT2 = wp.tile([64, M], F16, tag="kT2")
            qT2 = wp.tile([64, M], F16, tag="qT2")
            vaug = wp.tile([128, 16, HD + 1], BF16, tag="vaug")
            nc.vector.memset(vaug[:, :, HD:HD + 1], 1.0)
            attnT = wp.tile([128, 4, 256], F16, tag="attnT")
            attn_sb = wp.tile([128, 16, HD], F16, tag="attn_sb")
            zr = wp.tile([128, 16], F32, tag="zr")
            vacc = wp.tile([128, NS], F16, tag="vacc")
            ysb = wp.tile([128, 4, 256], F32, tag="ysb")

            # two explicit conv-input buffers; pad cells zeroed once
            kvsb = []
            for i in range(2):
                kb = wp.tile([128, NPAD], F16, tag=f"kvsb{i}",
                             name=f"kvsb{i}")
                nc.vector.memset(kb[:, :], 0.0)
                kvsb.append(kb)

            # PE warm-up: keep the array busy through the DMA window so the
            # p-state ramp completes before the real matmuls arrive.
            for _w in range(5):
                wmm = ps.tile([128, 512], F32, tag="kvp", bufs=1)
                nc.tensor.matmul(wmm[:, :], ident[:, :], kvsb[0][:, 0:512],
                                 start=True, stop=True)

            # ---------------- helpers ----------------
            def kv_proj_half(dg, nh):
                """kv proj for ch-tile dg, token half nh -> PSUM [128, 512]."""
                kvp = ps.tile([128, 512], F32, tag="kvp", bufs=1)
                for k in range(4):
                    nc.tensor.matmul(kvp[:, :],
                                     wkv[:, k, dg * 128:(dg + 1) * 128],
                                     xT[:, k, nh * 512:(nh + 1) * 512],
                                     start=(k == 0), stop=(k == 3))
                return kvp

            def pad_copy(dg, nh, kvp, eng):
                kb = kvsb[dg % 2]
                dst = bass.AP(tensor=kb.tensor,
                              offset=kb.offset + PADW * 16 * nh,
                              ap=[kb.ap[0], [PADW, 16], [1, 32]])
                src = kvp[:, :].rearrange("p (a b) -> p a b", b=32)
                if eng is nc.scalar:
                    eng.copy(dst, src)
                else:
                    eng.tensor_copy(dst, src)

            tmpp = tmppool

            def conv(dg):
                """Conv taps 0-6 on PE (PSUM cvp); taps 7,8 + bias into an
                SBUF side-accumulator (DVE then gpsimd), merged at the
                K/V stt."""
                kb = kvsb[dg % 2]
                tmp = tmpp.tile([128, NS], F32, tag="tmp")
                for tap, eng in ((7, nc.vector), (8, nc.vector)):
                    dy, dx = tap // 3, tap % 3
                    win = bass.AP(tensor=kb.tensor,
                                  offset=kb.offset + PADW * dy + dx,
                                  ap=[kb.ap[0], [2 * PADW, KH], [2, KW]])
                    col = 8 + (tap - 5) * 8 + dg
                    in1 = bpl[:, dg, :] if tap == 7 else tmp[:, :]
                    eng.scalar_tensor_tensor(
                        tmp[:, :], win, cst[:, col:col + 1], in1,
                        op0=ALU.mult, op1=ALU.add)
                cvp = ps.tile([128, NS], F32, tag="kvp", bufs=1)
                for tap in range(7):
                    dy, dx = tap // 3, tap % 3
                    win = bass.AP(tensor=kb.tensor,
                                  offset=kb.offset + PADW * dy + dx,
                                  ap=[kb.ap[0], [2 * PADW, KH], [2, KW]])
                    nc.tensor.matmul(cvp[:, :], dgw[:, dg * 7 + tap, :], win,
                                     start=(tap == 0), stop=(tap == 6))
                return cvp, tmp

            def k_tile(t, cvp, tmp):
                # gpsimd cannot read PSUM: merge to SBUF, then cross-copy
                kacc = tmpp.tile([128, NS], F16, tag="kacc")
                nc.vector.scalar_tensor_tensor(
                    kacc[:, :], cvp[:, :], 1.0, tmp[:, :],
                    op0=ALU.mult, op1=ALU.add)
                nc.gpsimd.tensor_copy(kT2[:, (2 * t) * 256:(2 * t + 1) * 256],
                                      kacc[0:64, :])
                nc.gpsimd.tensor_copy(
                    kT2[:, (2 * t + 1) * 256:(2 * t + 2) * 256],
                    kacc[64:128, :])

            def v_tile(t, cvp, tmp):
                nc.vector.scalar_tensor_tensor(
                    vacc[:, :], cvp[:, :], 1.0, tmp[:, :],
                    op0=ALU.mult, op1=ALU.add)
                for gi in range(2):
                    vt = ps.tile([128, 128], F16, tag="kvp", bufs=1)
                    for sh in range(2):
                        nc.tensor.transpose(
                            vt[:, sh * 64:(sh + 1) * 64],
                            vacc[gi * 64:(gi + 1) * 64,
                                 sh * 128:(sh + 1) * 128],
                            ident[gi * 64:(gi + 1) * 64,
                                  gi * 64:(gi + 1) * 64])
                    mt0 = 4 * t + 2 * gi
                    nc.vector.tensor_copy(
                        vaug[:, mt0:mt0 + 2, 0:HD],
                        vt[:, :].rearrange("p (a b) -> p a b", b=64))

            av = ps.tile([128, 16, HD + 1], F32, tag="av", bufs=1)
            # pre-zero the whole av region (PSUM zero-regions are 2KB; the
            # 16 interleaved 260B accumulation groups must share one group)
            av_flat = bass.AP(tensor=av.tensor, offset=av.offset,
                              ap=[av.ap[0], [1, 16 * (HD + 1)]])
            for z0 in (0, 512):
                zmm = bass.AP(tensor=av.tensor, offset=av.offset + z0,
                              ap=[av.ap[0], [1, 512]])
                nc.tensor.matmul(zmm, ident[:, :], kvsb[0][:, 0:512],
                                 start=True, stop=True, skip_group_check=True)
            zmm2 = bass.AP(tensor=av.tensor, offset=av.offset + 1024,
                           ap=[av.ap[0], [1, 16]])
            nc.tensor.matmul(zmm2, ident[:, :], kvsb[0][:, 0:16],
                             start=True, stop=True, skip_group_check=True)

            def sc_unit(mt, uq):
                st = ps.tile([128, 512], F32, tag="st", bufs=4)
                nc.tensor.matmul(st[:, :],
                                 kT2[:, mt * 128:(mt + 1) * 128],
                                 qT2[:, uq * 512:(uq + 1) * 512],
                                 start=True, stop=True)
                ex = expp.tile([128, 512], BF16, tag="ex")
                nc.scalar.activation(ex[:, :], st[:, :], AF.Exp,
                                     scale=float(SCALE))
                return ex

            def av_unit(mt, uq, ex):
                for j in range(4):
                    nc.tensor.matmul(av[:, uq * 4 + j, :],
                                     ex[:, j * 128:(j + 1) * 128],
                                     vaug[:, mt, :],
                                     start=False, stop=(mt == 15),
                                     skip_group_check=True)

            # ---------------- q projection (emitted via emit_qproj) --------
            def emit_qproj(t):
                qp = ps.tile([128, 512], F32, tag="st", bufs=4)
                qpv = qp[:, 0:256]
                for k in range(4):
                    nc.tensor.matmul(qpv, wq[:, k, t * 128:(t + 1) * 128],
                                     xTc[:, k, :],
                                     start=(k == 0), stop=(k == 3))
                nc.vector.tensor_scalar_add(qpv, qpv, cst[:, t:t + 1])
                nc.vector.tensor_copy(qT2[:, (2 * t) * 256:(2 * t + 1) * 256],
                                      qp[0:64, 0:256])
                qstage = tmpp.tile([128, 256], F16, tag="kacc", name="qstage")
                nc.vector.tensor_copy(qstage[64:128, :], qp[64:128, 0:256])
                nc.gpsimd.tensor_copy(qT2[:, (2 * t + 1) * 256:(2 * t + 2) * 256],
                                      qstage[64:128, :])

            # ---------------- fused P1/P2 pipeline ----------------
            # K-proj halves rotate through the "st" slots (parallel to the
            # V chain on "kvp"); t=0 pads on ACT (idle pre-exp), later K
            # pads on DVE, V pads on gpsimd.
            kvp_store = {}

            def pad_eng(dg):
                # gpsimd cannot access PSUM; split PSUM reads DVE/ACT
                if dg < 2:
                    return nc.scalar
                return nc.vector

            def proj_op(dg, nh):
                if dg % 2 == 0:  # K tile -> st slots
                    kvp = ps.tile([128, 512], F32, tag="st", bufs=4,
                                  name="kvpk")
                else:
                    kvp = ps.tile([128, 512], F32, tag="kvp", bufs=1)
                for k in range(4):
                    nc.tensor.matmul(kvp[:, :],
                                     wkv[:, k, dg * 128:(dg + 1) * 128],
                                     xT[:, k, nh * 512:(nh + 1) * 512],
                                     start=(k == 0), stop=(k == 3))
                kvp_store[(dg, nh)] = kvp

            def pad_op(dg, nh):
                pad_copy(dg, nh, kvp_store.pop((dg, nh)), pad_eng(dg))

            def p1_ops(t):
                dgk, dgv = 2 * t, 2 * t + 1
                mk = lambda f, *a: (lambda: f(*a))
                cstore = {}

                def conv_op(dg):
                    cstore[dg] = conv(dg)

                def ktail(tt):
                    k_tile(tt, *cstore.pop(2 * tt))

                def vtail(tt):
                    v_tile(tt, *cstore.pop(2 * tt + 1))

                return [
                    mk(proj_op, dgk, 0), mk(pad_op, dgk, 0),
                    mk(proj_op, dgv, 0), mk(pad_op, dgv, 0),
                    mk(proj_op, dgk, 1), mk(pad_op, dgk, 1),
                    mk(proj_op, dgv, 1), mk(pad_op, dgv, 1),
                    mk(conv_op, dgk), mk(ktail, t),
                    mk(conv_op, dgv), mk(vtail, t),
                ]

            ops0 = p1_ops(0)
            for op in ops0[0:4]:     # K0/V0 first halves
                op()
            emit_qproj(0)
            for op in ops0[4:8]:     # second halves
                op()
            emit_qproj(1)
            emit_qproj(2)
            for op in ops0[8:10]:    # conv K0
                op()
            emit_qproj(3)
            for op in ops0[10:12]:   # conv V0
                op()
            # Unit stream with av lagging sc by one; p1 micro-ops of tile
            # t+1 round-robined between tile t's units.
            pending = []
            for t in range(4):
                us = [(mt, uq) for mt in range(4 * t, 4 * t + 4)
                      for uq in range(4)]
                chunks = p1_ops(t + 1) if t < 3 else []
                nu, nch = len(us), len(chunks)
                ci = 0
                for i, u in enumerate(us):
                    mt, uq = u
                    ex = sc_unit(mt, uq)
                    pending.append((mt, uq, ex))
                    if len(pending) > 2:
                        av_unit(*pending.pop(0))
                    want = ((i + 1) * nch) // nu
                    while ci < want:
                        chunks[ci]()
                        ci += 1
            for p_ in pending:
                av_unit(*p_)

            # ---------------- normalize + attnT + y proj ----------------
            # chunks 0-7 (half 0) complete one unit before 8-15; normalize,
            # transpose and accumulate y per ch-block as results land.
            def slot_of(chunk):
                h, lh = chunk // 2, chunk % 2
                return 4 * (h // 2) + 2 * lh + (h % 2)

            def norm_chunk(chunk):
                slot = slot_of(chunk)
                nc.vector.reciprocal(zr[:, chunk:chunk + 1],
                                     av[:, chunk, HD:HD + 1])
                if chunk % 2 == 0:
                    nc.vector.tensor_scalar_mul(attn_sb[:, slot, :],
                                                av[:, chunk, 0:HD],
                                                zr[:, chunk:chunk + 1])
                else:
                    nc.scalar.mul(attn_sb[:, slot, :], av[:, chunk, 0:HD],
                                  zr[:, chunk:chunk + 1])

            yps = [None, None]

            def attnT_block(kk):
                for lh in range(2):
                    vt2 = ps.tile([128, 128], F16, tag="kvp", bufs=1)
                    s0 = 4 * kk + 2 * lh
                    src_ap = bass.AP(
                        tensor=attn_sb.tensor,
                        offset=attn_sb.offset + s0 * HD,
                        ap=[attn_sb.ap[0], [1, 128]])
                    nc.tensor.transpose(vt2[:, :], src_ap, ident[:, :])
                    if kk < 2:
                        nc.scalar.copy(
                            attnT[:, kk, lh * 128:(lh + 1) * 128], vt2[:, :])
                    else:
                        nc.vector.tensor_copy(
                            attnT[:, kk, lh * 128:(lh + 1) * 128], vt2[:, :])
                for m in range(4):
                    if yps[m // 2] is None:
                        yps[m // 2] = ps.tile([128, 512], F32, tag="st",
                                              bufs=4, name=f"yp{m // 2}")
                    nc.tensor.matmul(
                        yps[m // 2][:, (m % 2) * 256:(m % 2) * 256 + 256],
                        wo[:, kk, m * 128:(m + 1) * 128],
                        attnT[:, kk, :],
                        start=(kk == 0 and m % 2 == 0), stop=(kk == 3),
                        skip_group_check=True)

            for chunk in range(8):
                norm_chunk(chunk)
            attnT_block(0)
            attnT_block(1)
            for chunk in range(8, 16):
                norm_chunk(chunk)
            attnT_block(2)
            attnT_block(3)
            for m in range(4):
                ypv = yps[m // 2][:, (m % 2) * 256:(m % 2) * 256 + 256]
                if m < 2:
                    nc.vector.tensor_scalar_add(ysb[:, m, :], ypv,
                                                cst[:, 4 + m:5 + m])
                else:
                    nc.scalar.add(ysb[:, m, :], ypv, cst[:, 4 + m:5 + m])
            if _dbg:
                nc.sync.dma_start(out=kT2_o[:, :], in_=kT2[:, :])
                nc.sync.dma_start(out=qT2_o[:, :], in_=qT2[:, :])
                nc.sync.dma_start(out=vaug_o[:, :],
                                  in_=vaug[:, :, :].rearrange("p a b -> p (a b)"))
                nc.sync.dma_start(out=asb_o[:, :],
                                  in_=attn_sb[:, :, :].rearrange("p a b -> p (a b)"))
            y_r = y_d[:, :].rearrange("(m p) t -> p m t", p=128)
            nc.sync.dma_start(out=y_d[0:256, :].rearrange("(m p) t -> p m t", p=128),
                              in_=ysb[:, 0:2, :])
            nc.sync.dma_start(out=y_d[256:512, :].rearrange("(m p) t -> p m t", p=128),
                              in_=ysb[:, 2:4, :])

    nc.finalize()
    return nc


def _get_program():
    if "nc" not in _NC_CACHE:
        _NC_CACHE["nc"] = _build_program()
    return _NC_CACHE["nc"]


def _host_prep(x, wq, bq, wkv, bkv, dw_kernel, dw_bias, wo, bo):
    """Build the 8 per-core input maps."""
    x = np.ascontiguousarray(np.asarray(x, np.float32))
    wq16 = np.asarray(wq, np.float32).astype(np.float16)
    wo16 = np.asarray(wo, np.float32).astype(np.float16)
    bq = np.asarray(bq, np.float32)
    bkv = np.asarray(bkv, np.float32)
    dw_bias = np.asarray(dw_bias, np.float32)
    bo = np.asarray(bo, np.float32)
    dww = np.asarray(dw_kernel, np.float32).reshape(9, CH).T.copy()  # [1024, 9]

    # channel-tile processing order K0 V0 K1 V1 ... ; dg -> channel base
    dg_base = []
    for t in range(4):
        dg_base += [t * 128, DIM + t * 128]

    # wkv columns reordered to dg order
    wkv_f = np.asarray(wkv, np.float32)
    wkv16 = np.empty((DIM, CH), np.float16)
    for dg in range(8):
        b0 = dg_base[dg]
        wkv16[:, dg * 128:(dg + 1) * 128] = wkv_f[:, b0:b0 + 128]

    # bias plane: dw_bias + bkv * sum(valid taps), SAME padding aware,
    # rows in dg order
    oy = np.arange(KH)
    valid_y = (2 * oy[:, None] + np.arange(3)[None, :]) < H      # [16, 3]
    valid_x = valid_y.copy()
    wsum = np.zeros((CH, KH, KW), np.float32)
    for tap in range(9):
        dy, dx = tap // 3, tap % 3
        m2 = np.outer(valid_y[:, dy], valid_x[:, dx]).astype(np.float32)
        wsum += dww[:, tap][:, None, None] * m2[None, :, :]
    bpl_full = (dw_bias[:, None] + bkv[:, None] * wsum.reshape(CH, NS))
    bpl16 = np.empty((CH, NS), np.float16)
    for dg in range(8):
        b0 = dg_base[dg]
        bpl16[dg * 128:(dg + 1) * 128] = bpl_full[b0:b0 + 128]

    # conv diagonal weights [128, 72*128] fp16, dg-major then tap
    dgw = np.zeros((128, 56, 128), np.float16)
    idx = np.arange(128)
    for dg in range(8):
        b0 = dg_base[dg]
        for tap in range(7):
            dgw[idx, dg * 7 + tap, idx] = dww[b0 + idx, tap].astype(np.float16)
    dgw = dgw.reshape(128, 56 * 128)

    cst = np.zeros((128, 40), np.float32)
    cst[:, 0:4] = bq.reshape(4, 128).T
    cst[:, 4:8] = bo.reshape(4, 128).T
    for dg in range(8):
        b0 = dg_base[dg]
        for tap in range(5, 9):
            cst[:, 8 + (tap - 5) * 8 + dg] = dww[b0:b0 + 128, tap]

    in_maps = []
    for c in range(8):
        b, j = c // 4, c % 4
        xT = x[b].reshape(L, DIM).T.astype(np.float16)
        in_maps.append({
            "xT": np.ascontiguousarray(xT),
            "xTc": np.ascontiguousarray(xT[:, j * 256:(j + 1) * 256]),
            "wkv": wkv16, "wq": wq16, "wo": wo16,
            "bpl": bpl16, "dgw": dgw, "cst": cst,
        })
    return in_maps


def kernel(**inputs) -> np.ndarray:
    nc = _get_program()
    in_maps = _host_prep(**inputs)
    res = run_bass_kernel_spmd(nc, in_maps, core_ids=list(range(8)))
    out = np.zeros((B, H, W, DIM), np.float32)
    flat = out.reshape(B, L, DIM)
    for c in range(8):
        b, j = c // 4, c % 4
        flat[b, j * 256:(j + 1) * 256, :] = res.results[c]["y"].T
    return out
